# revision 11
# baseline (speedup 1.0000x reference)
"""CrossEncoderReranker TRN2 Bass kernel (v2).

reference computation:
    x = concat([mention_embs[mention_idx], candidate_embs], 1)   # [T, 2H]
    h = relu(x @ W1 + b1)                                        # [T, H]
    s = (h @ W2 + b2)[:, 0]                                      # [T]
    out = scatter(s -> [N, MAXK] at (mention_idx, col_idx)) + 0.5 * faiss
    out = concat([out, nota_col], 1)                             # [N, MAXK+1]

Device strategy (8-way data parallel over mentions):
  * The generator's ragged lengths pair up to exactly 64 (32+d with 32-d),
    so mentions are matched into pairs and dealt 256 pairs per core:
    every core gets exactly 512 mentions / 16384 candidates, and every
    128-mention block exactly 4096 candidates.  No padding, NT=32 tiles,
    MC=4 blocks, and each candidate tile maps to exactly one block.
  * A = mention_embs @ W1_top is computed on the HOST (tiny GEMM) and fed
    per-core in bf16; the device adds it into the pre-relu PSUM via one
    one-hot "expansion" matmul per (jc, tile) (E is host-built, packed
    into the same DMA slab as the candidate chunks).
  * Everything on the matmul path is bf16 (tolerance 2e-2; bf16 ~4e-3):
    halves HBM traffic and the startup DMA wait.
  * relu(psum + b1[j]) on ACT -> bf16 ht; W2 reduction on PE (6 matmuls
    of [128,1] weights); W2(jc) is issued after jc+1's matmuls so the PE
    never waits on ACT.
  * scores -> DRAM scratch with per-block 64-elem zero tails, so each
    output block's ragged->padded gather (indirect DMA windows, masked,
    + 0.5*faiss, both host-precomputed) depends only on its own chunk.
  * A short burst of warm-up matmuls runs during the initial DMA wait to
    lift the HAM clock gate (PE starts at 1.2 GHz otherwise).
"""

import sys

sys.path.insert(0, "/opt/trn_rl_repo")

from contextlib import ExitStack

import numpy as np
import ml_dtypes

import concourse.bass as bass
import concourse.tile as tile
from concourse import mybir
from concourse.tile_rust import add_dep_helper

F32 = mybir.dt.float32
BF16 = mybir.dt.bfloat16
I32 = mybir.dt.int32
AF = mybir.ActivationFunctionType
ALU = mybir.AluOpType
BF16NP = ml_dtypes.bfloat16

N_CORES = 8
H = 768
P = 128
KC = H // P            # 6 k-chunks per 768
JC = H // P            # 6 j-chunks
MAXK = 64
TT = 512               # candidate tile size
N_WARM = 40            # warm-up matmuls (~3.6us) to lift the HAM clock gate


class SplitDrainTileContext(tile.TileContext):
    """The tail drain would carry one sync wait per logical proc; walrus caps
    sync waits per instruction. Absorb the global clock one proc at a time
    through SP NOPs (<=1 wait each), then emit the drain with a zero clock."""

    def _drain_and_barrier(self, tick_clock, wait_clock):
        from concourse.vector_clock import ScopedClock, VectorClock

        vals = list(tick_clock.global_clock)
        nprocs = len(vals)
        for q in range(nprocs):
            if not vals[q]:
                continue
            partial = [vals[p] if p == q else 0 for p in range(nprocs)]
            nop = self.nc.sync.nop()
            wait_clock.add_sem_waits(
                nop.ins, ScopedClock({None: VectorClock(partial)})
            )
        drain_inst = self.nc.sync.drain()
        wait_clock.add_sem_waits(
            drain_inst.ins, ScopedClock({None: VectorClock([0] * nprocs)})
        )
        self.nc.all_engine_barrier()
        popped = self.nc._tile_sem_poison_stack.pop()
        assert popped is self._sem_poison
        self.nc.clear_and_free_semaphores(list(self.sems.allocated().values()))
        self.nc.all_engine_barrier()


def split_waits(nc, cap=1):
    """This walrus build allows only ONE sync wait per instruction (two for
    some structs, but one is universally safe).  Move extra waits onto
    freshly inserted same-engine NOPs placed right before the instruction —
    the engine stalls at the NOP instead, semantics unchanged."""
    for fn in nc.m.functions:
        for bb in fn.blocks:
            new = []
            for inst in bb.instructions:
                si = inst.sync_info
                waits = list(si.on_wait) if si and si.on_wait else []
                if len(waits) > cap:
                    keep = waits[-cap:]
                    for k, wt in enumerate(waits[:-cap]):
                        nop = mybir.InstNoOp(
                            name=f"{inst.name}-wsp{k}",
                            engine=inst.engine,
                            ins=[], outs=[],
                            sync_info=mybir.SyncInfo(on_wait=[wt], on_update=[]),
                        )
                        nc.register_instruction(nop)
                        new.append(nop)
                    inst.sync_info = mybir.SyncInfo(
                        on_wait=keep, on_update=list(si.on_update or [])
                    )
                new.append(inst)
            bb.instructions = new


def build_program(NT, MC, windows, gdep, aligned, T_pad):
    """One SPMD Bass program shared by all cores.

    windows[i]: sorted local m-chunk ids present in candidate tile i on ANY
    core (union).  gdep[mc]: scores-chunk index that must land before output
    block mc can be gathered.  aligned=True means chunk==block with per-block
    zero tails in the scratch.
    """
    assert len(windows) == NT
    assert len(gdep) == MC
    CHT = NT // MC if aligned else 8
    n_chunks = -(-NT // CHT)
    CH = CHT * TT
    stride = CH + 64 if aligned else CH
    slen = (n_chunks - 1) * stride + CH + 64 if aligned else T_pad + MAXK
    slab_cols = [(KC + len(windows[i])) * TT for i in range(NT)]
    slab_base = np.concatenate([[0], np.cumsum(slab_cols)]).astype(int)

    nc = bass.Bass()

    candE = nc.dram_tensor("candE", [P, int(slab_base[-1])], BF16,
                           kind="ExternalInput")
    w1b = nc.dram_tensor("w1b", [P, KC * H], BF16, kind="ExternalInput")
    a_t = nc.dram_tensor("a", [P, MC * H], BF16, kind="ExternalInput")
    w2 = nc.dram_tensor("w2", [P, JC], BF16, kind="ExternalInput")
    b1 = nc.dram_tensor("b1", [P, JC], F32, kind="ExternalInput")
    b2 = nc.dram_tensor("b2", [1, 1], F32, kind="ExternalInput")
    warm = nc.dram_tensor("warm", [1, P], BF16, kind="ExternalInput")
    offs = nc.dram_tensor("offs", [P, MC], I32, kind="ExternalInput")
    maskf = nc.dram_tensor("maskf", [P, MC * MAXK], F32, kind="ExternalInput")
    fh = nc.dram_tensor("fh", [P, MC * MAXK], F32, kind="ExternalInput")

    out = nc.dram_tensor("out", [MC * P, MAXK], F32, kind="ExternalOutput")
    sc_dram = nc.dram_tensor("sc_scratch", [slen, 1], F32, kind="Internal")
    sc_flat = sc_dram[:].rearrange("t a -> (a t)")[None, :]

    with ExitStack() as ctx:
        tc = ctx.enter_context(SplitDrainTileContext(nc))
        cst = ctx.enter_context(tc.tile_pool(name="cst", bufs=1))
        candp = ctx.enter_context(tc.tile_pool(name="candp", bufs=4))
        htp = ctx.enter_context(tc.tile_pool(name="htp", bufs=4))
        gp = ctx.enter_context(tc.tile_pool(name="gp", bufs=2))
        scp = ctx.enter_context(tc.tile_pool(name="scp", bufs=2))
        hps = ctx.enter_context(tc.tile_pool(name="hps", bufs=5, space="PSUM"))
        sps = ctx.enter_context(tc.tile_pool(name="sps", bufs=2, space="PSUM"))
        wps_pool = ctx.enter_context(
            tc.tile_pool(name="wps", bufs=1, space="PSUM"))

        # ---- constants; spread startup DMAs over the three DGE queues
        # (SP + ACT + GPSIMD) so the first tile's inputs land ASAP ----
        warm_sb = cst.tile([1, P], BF16)
        nc.sync.dma_start(warm_sb[:], warm[:])
        w1b_sb = cst.tile([P, KC * H], BF16)
        half = KC * H // 2
        nc.sync.dma_start(w1b_sb[:, 0:half], w1b[0:P, 0:half])
        nc.scalar.dma_start(w1b_sb[:, half:], w1b[0:P, half:])
        a_sb = cst.tile([P, MC * H], BF16)
        nc.scalar.dma_start(a_sb[:], a_t[:])
        w2_sb = cst.tile([P, JC], BF16)
        nc.scalar.dma_start(w2_sb[:], w2[:])
        b1_sb = cst.tile([P, JC], F32)
        nc.scalar.dma_start(b1_sb[:], b1[:])
        b2_sb = cst.tile([1, 1], F32)
        nc.scalar.dma_start(b2_sb[:], b2[:])
        offs_sb = cst.tile([P, MC], I32)
        nc.gpsimd.dma_start(offs_sb[:], offs[:])
        maskf_sb = cst.tile([P, MC * MAXK], F32)
        nc.gpsimd.dma_start(maskf_sb[:], maskf[:])
        fh_sb = cst.tile([P, MC * MAXK], F32)
        nc.gpsimd.dma_start(fh_sb[:], fh[:])

        # zero tails of the scores scratch (gather windows overread 64)
        z_t = cst.tile([1, MAXK], F32)
        nc.vector.memset(z_t[:], 0.0)
        zero_dmas = []
        if aligned:
            for c in range(n_chunks):
                zero_dmas.append(nc.gpsimd.dma_start(
                    sc_flat[0:1, c * stride + CH:c * stride + CH + MAXK],
                    z_t[0:1, :]))
        else:
            zero_dmas.append(nc.gpsimd.dma_start(
                sc_flat[0:1, T_pad:T_pad + MAXK], z_t[0:1, :]))

        def dummy_ldw(src_ap, dep_of=None):
            """1-elem bf16 ldweights — absorbs a cross-engine wait into the
            PE stream early, so the real matmul carries no sync wait."""
            ap = src_ap[0:1, 0:1]
            if ap.dtype != BF16:
                ap = ap.bitcast(BF16)
            d = nc.tensor.ldweights(ap)
            if dep_of is not None:
                add_dep_helper(d.ins, dep_of.ins, reason="absorb wait")
            return d

        # ---- warm-up spinner: lift the HAM clock gate during DMA wait ----
        wps = wps_pool.tile([P, P], F32, tag="warmps")
        for k in range(N_WARM):
            nc.tensor.matmul(
                wps[:], lhsT=warm_sb[:], rhs=warm_sb[:],
                start=(k == 0), stop=(k == N_WARM - 1),
            )

        # ---- output stage: gather + mask + faiss for block mc ----
        def emit_out_chunk(mc, dep_dmas):
            g_t = gp.tile([P, MAXK], F32, tag="gath")
            gth = nc.gpsimd.indirect_dma_start(
                out=g_t[:], out_offset=None,
                in_=sc_dram[:],
                in_offset=bass.IndirectOffsetOnAxis(
                    ap=offs_sb[:, mc:mc + 1], axis=0),
            )
            for d in dep_dmas:
                add_dep_helper(gth.ins, d.ins, reason="gather needs scores")
            gm_t = gp.tile([P, MAXK], F32, tag="gm")
            nc.vector.tensor_tensor(
                gm_t[:], g_t[:], maskf_sb[:, mc * MAXK:(mc + 1) * MAXK],
                ALU.mult)
            o_t = gp.tile([P, MAXK], F32, tag="osb")
            nc.vector.tensor_tensor(
                o_t[:], gm_t[:], fh_sb[:, mc * MAXK:(mc + 1) * MAXK], ALU.add)
            nc.sync.dma_start(out[mc * P:(mc + 1) * P, :], o_t[:])

        # ---- main loop ----
        # wait-absorption state: last consumer of each recycled slot, so the
        # slot's next producer carries no sync wait (a matmul/ldweights with
        # an attached wait costs ~+100ns on the PE even when satisfied)
        NHPS = 5
        hps_relu = [None] * NHPS      # relu that freed each hps slot
        hps_ctr = [0]
        sps_copy = [None, None]       # score-copy that freed each sps slot
        sc_t = None
        pend_w2 = None                # deferred W2 matmul args
        pend_sc = None                # deferred score-copy args

        def flush_w2():
            nonlocal pend_w2
            if pend_w2 is not None:
                s_ps, jc, ht_t, relu_i = pend_w2
                dummy_ldw(ht_t, dep_of=relu_i)
                nc.tensor.matmul(
                    s_ps[0:1, :], lhsT=w2_sb[:, jc:jc + 1], rhs=ht_t[:],
                    start=(jc == 0), stop=(jc == JC - 1),
                )
                pend_w2 = None

        def flush_sc():
            nonlocal pend_sc
            if pend_sc is not None:
                s_ps, i = pend_sc
                cp = nc.scalar.activation(
                    sc_t[0:1, (i % CHT) * TT:(i % CHT) * TT + TT],
                    s_ps[0:1, :], AF.Identity, bias=b2_sb[0:1, 0:1],
                )
                sps_copy[i % 2] = cp
                pend_sc = None

        for i in range(NT):
            if i % CHT == 0:
                sc_t = scp.tile([1, CH], F32, tag="scchunk")
            cand_t = candp.tile([P, slab_cols[i]], BF16, tag="cand")
            cdma = nc.sync.dma_start(
                cand_t[:], candE[0:P, int(slab_base[i]):int(slab_base[i + 1])]
            )
            dummy_ldw(cand_t, dep_of=cdma)      # absorb the slab-DMA wait
            if sps_copy[i % 2] is not None:     # absorb sps slot-free wait
                dummy_ldw(w1b_sb, dep_of=sps_copy[i % 2])
            s_ps = sps.tile([1, TT], F32, tag="spsum")
            for jc in range(JC):
                slot = hps_ctr[0] % NHPS
                hps_ctr[0] += 1
                if hps_relu[slot] is not None:  # absorb hps slot-free wait
                    dummy_ldw(w1b_sb, dep_of=hps_relu[slot])
                ps = hps.tile([P, TT], F32, tag="hpsum")
                for kc in range(KC):
                    nc.tensor.matmul(
                        ps[:],
                        lhsT=w1b_sb[:, kc * H + jc * P:kc * H + (jc + 1) * P],
                        rhs=cand_t[:, kc * TT:(kc + 1) * TT],
                        start=(kc == 0), stop=False,
                    )
                nw = len(windows[i])
                for wi, w in enumerate(windows[i]):
                    nc.tensor.matmul(
                        ps[:],
                        lhsT=a_sb[:, w * H + jc * P:w * H + (jc + 1) * P],
                        rhs=cand_t[:, (KC + wi) * TT:(KC + wi + 1) * TT],
                        start=False, stop=(wi == nw - 1),
                    )
                # interleave: previous jc's W2 runs now (its relu is done),
                # previous tile's score copy after this tile's first group
                flush_w2()
                if jc == 1:
                    flush_sc()
                ht_t = htp.tile([P, TT], BF16, tag="ht")
                relu_i = nc.scalar.activation(
                    ht_t[:], ps[:], AF.Relu, bias=b1_sb[:, jc:jc + 1]
                )
                hps_relu[slot] = relu_i
                pend_w2 = (s_ps, jc, ht_t, relu_i)
            pend_sc = (s_ps, i)

            if i % CHT == CHT - 1 or i == NT - 1:
                flush_w2()
                flush_sc()
                ci = i // CHT
                c0 = ci * stride
                cn = min(CH, T_pad - ci * CH)
                d = nc.sync.dma_start(
                    sc_flat[0:1, c0:c0 + cn], sc_t[0:1, 0:cn]
                )
                for mc in range(MC):
                    if gdep[mc] == ci:
                        emit_out_chunk(mc, [d] + zero_dmas)

    split_waits(nc)
    return nc


def _pair_mentions(lengths):
    """Match mentions into pairs with length sum exactly 64 (the generator
    pairs 32+d with 32-d).  Returns [n_pairs, 2] global ids or None."""
    n = len(lengths)
    if n % 2:
        return None
    order = np.argsort(lengths, kind="stable")
    lo, hi = 0, n - 1
    pairs = []
    while lo < hi:
        a, b = order[lo], order[hi]
        if lengths[a] + lengths[b] != 64:
            return None
        pairs.append((a, b))
        lo += 1
        hi -= 1
    return np.asarray(pairs, dtype=np.int64)


def prepare(inputs):
    """Shard + lay out the full inputs; returns (build params, in_maps, meta)."""
    mention_embs = np.asarray(inputs["mention_embs"], dtype=np.float32)
    candidate_embs = np.asarray(inputs["candidate_embs"], dtype=np.float32)
    W1 = np.asarray(inputs["W1"], dtype=np.float32)
    b1 = np.asarray(inputs["b1"], dtype=np.float32)
    W2 = np.asarray(inputs["W2"], dtype=np.float32)
    b2 = np.asarray(inputs["b2"], dtype=np.float32)
    faiss_prior = np.asarray(inputs["faiss_prior"], dtype=np.float32)
    mention_idx = np.asarray(inputs["mention_idx"], dtype=np.int64)
    col_idx = np.asarray(inputs["col_idx"], dtype=np.int64)

    N = mention_embs.shape[0]
    T = mention_idx.shape[0]
    assert np.all(np.diff(mention_idx) >= 0), "mention_idx must be sorted"
    lengths = np.bincount(mention_idx, minlength=N)
    offsets = np.concatenate([[0], np.cumsum(lengths)[:-1]])
    assert np.array_equal(col_idx, np.arange(T) - np.repeat(offsets, lengths))

    pairs = _pair_mentions(lengths) if (N % (2 * N_CORES) == 0) else None
    if pairs is not None:
        # perfect split: 256 pairs -> 512 mentions / 16384 cands per core,
        # every 128 mentions (64 pairs) = exactly 4096 candidates
        ppc = pairs.shape[0] // N_CORES
        perms = [pairs[c * ppc:(c + 1) * ppc].reshape(-1)
                 for c in range(N_CORES)]
        aligned = True
    else:
        # fallback: greedy balance by candidate count
        order = np.argsort(-lengths, kind="stable")
        loads = np.zeros(N_CORES, dtype=np.int64)
        buckets = [[] for _ in range(N_CORES)]
        for m in order:
            c = int(np.argmin(loads))
            buckets[c].append(m)
            loads[c] += lengths[m]
        perms = [np.asarray(sorted(b), dtype=np.int64) for b in buckets]
        aligned = False

    T_cs = [int(lengths[p].sum()) for p in perms]
    M_cs = [len(p) for p in perms]
    T_pad = -(-max(T_cs) // TT) * TT
    M_pad = -(-max(M_cs) // P) * P
    NT, MC = T_pad // TT, M_pad // P
    CHT = NT // MC if aligned else 8
    n_chunks = -(-NT // CHT)
    CH = CHT * TT
    stride = CH + 64 if aligned else CH

    # host-side mention-part GEMM (tiny): A = mention_embs @ W1_top
    A = mention_embs @ W1[:H]

    windows = [set() for _ in range(NT)]
    core_data = []
    for c in range(N_CORES):
        perm = perms[c]
        T_c, M_c = T_cs[c], M_cs[c]
        lens_c = lengths[perm]
        offs_c = np.concatenate([[0], np.cumsum(lens_c)[:-1]])
        ml = np.full(T_pad, -1, dtype=np.int64)
        ml[:T_c] = np.repeat(np.arange(M_c), lens_c)
        for i in range(NT):
            seg = ml[i * TT:(i + 1) * TT]
            seg = seg[seg >= 0]
            if seg.size:
                for w in np.unique(seg // P):
                    windows[i].add(int(w))
        core_data.append((perm, T_c, M_c, lens_c, offs_c, ml))
    windows = [sorted(w) if w else [0] for w in windows]

    if aligned:
        gdep = list(range(MC))
        assert windows == [[i // CHT] for i in range(NT)]
    else:
        gdep = [0] * MC
        for c in range(N_CORES):
            perm, T_c, M_c, lens_c, offs_c, ml = core_data[c]
            for mc in range(MC):
                rows = offs_c[mc * P:(mc + 1) * P]
                if rows.size == 0:
                    continue
                end = min(int(rows.max()) + MAXK, T_pad)
                k = min((end - 1) // CH, n_chunks - 1)
                gdep[mc] = max(gdep[mc], k)

    slab_cols = [(KC + len(windows[i])) * TT for i in range(NT)]
    slab_base = np.concatenate([[0], np.cumsum(slab_cols)]).astype(int)

    # shared (replicated) tensors
    w1b_l = np.ascontiguousarray(
        W1[H:].reshape(KC, P, H).transpose(1, 0, 2).reshape(P, KC * H)
    ).astype(BF16NP)
    w2_l = np.ascontiguousarray(W2[:, 0].reshape(JC, P).T).astype(BF16NP)
    b1_l = np.ascontiguousarray(b1.reshape(JC, P).T)
    b2_l = b2.reshape(1, 1)
    warm_l = np.ones((1, P), dtype=BF16NP)
    iota64 = np.arange(MAXK, dtype=np.float32)[None, :]

    in_maps = []
    for c in range(N_CORES):
        perm, T_c, M_c, lens_c, offs_c, ml = core_data[c]
        # gather this core's candidate rows in core-local order
        sel = (np.repeat(offsets[perm] - offs_c, lens_c)
               + np.arange(T_c)) if M_c else np.zeros(0, dtype=np.int64)
        cand_core = candidate_embs[sel]                      # [T_c, H] f32
        candT = np.zeros((P, KC, T_pad), dtype=BF16NP)
        candT[:, :, :T_c] = cand_core.astype(BF16NP).T.reshape(
            KC, P, T_c).transpose(1, 0, 2)

        candE_l = np.zeros((P, int(slab_base[-1])), dtype=BF16NP)
        for i in range(NT):
            b0 = int(slab_base[i])
            candE_l[:, b0:b0 + KC * TT] = candT[
                :, :, i * TT:(i + 1) * TT].reshape(P, KC * TT)
            seg = ml[i * TT:(i + 1) * TT]
            for wi, w in enumerate(windows[i]):
                e = (seg[None, :] ==
                     (w * P + np.arange(P))[:, None]).astype(BF16NP)
                candE_l[:, b0 + (KC + wi) * TT:b0 + (KC + wi + 1) * TT] = e

        A_core = np.zeros((MC * P, H), dtype=np.float32)
        A_core[:M_c] = A[perm]
        a_l = np.ascontiguousarray(
            A_core.reshape(MC, P, H).transpose(1, 0, 2).reshape(P, MC * H)
        ).astype(BF16NP)

        offs_l = np.zeros(MC * P, dtype=np.int64)
        offs_l[:M_c] = offs_c
        if aligned:
            # per-block scratch regions are (CH + 64) apart
            offs_l[:M_c] = offs_c + 64 * (np.arange(M_c) // P)
        lens_l = np.zeros(MC * P, dtype=np.int64)
        lens_l[:M_c] = lens_c
        maskf_l = (iota64 < lens_l[:, None]).astype(np.float32)
        fh_l = np.zeros((MC * P, MAXK), dtype=np.float32)
        fh_l[:M_c] = 0.5 * faiss_prior[perm]

        in_maps.append({
            "candE": candE_l,
            "w1b": w1b_l, "a": a_l, "w2": w2_l,
            "b1": b1_l, "b2": b2_l, "warm": warm_l,
            "offs": np.ascontiguousarray(
                offs_l.reshape(MC, P).T).astype(np.int32),
            "maskf": np.ascontiguousarray(
                maskf_l.reshape(MC, P, MAXK).transpose(1, 0, 2)
                .reshape(P, MC * MAXK)),
            "fh": np.ascontiguousarray(
                fh_l.reshape(MC, P, MAXK).transpose(1, 0, 2)
                .reshape(P, MC * MAXK)),
        })
    return (NT, MC, windows, gdep, aligned, T_pad), in_maps, (perms, N)


def assemble(results, meta, nota_bias):
    perms, N = meta
    out = np.empty((N, MAXK + 1), dtype=np.float32)
    for c in range(N_CORES):
        out[perms[c], :MAXK] = results[c]["out"][:len(perms[c])]
    out[:, MAXK] = np.float32(nota_bias)
    return out


_CACHE = {}


def kernel(**inputs) -> np.ndarray:
    from concourse.bass_utils import run_bass_kernel_spmd

    key_params, in_maps, meta = prepare(inputs)
    NT, MC, windows, gdep, aligned, T_pad = key_params
    key = (NT, MC, tuple(tuple(w) for w in windows), tuple(gdep), aligned,
           T_pad)
    if key not in _CACHE:
        _CACHE[key] = build_program(NT, MC, windows, gdep, aligned, T_pad)
    nc = _CACHE[key]
    res = run_bass_kernel_spmd(nc, in_maps, list(range(N_CORES)))
    return assemble(res.results, meta, np.asarray(inputs["nota_bias"]))


# revision 18
# speedup vs baseline: 1.3116x; 1.3116x over previous
"""CrossEncoderReranker TRN2 Bass kernel (v2).

reference computation:
    x = concat([mention_embs[mention_idx], candidate_embs], 1)   # [T, 2H]
    h = relu(x @ W1 + b1)                                        # [T, H]
    s = (h @ W2 + b2)[:, 0]                                      # [T]
    out = scatter(s -> [N, MAXK] at (mention_idx, col_idx)) + 0.5 * faiss
    out = concat([out, nota_col], 1)                             # [N, MAXK+1]

Device strategy (8-way data parallel over mentions):
  * The generator's ragged lengths pair up to exactly 64 (32+d with 32-d),
    so mentions are matched into pairs and dealt 256 pairs per core:
    every core gets exactly 512 mentions / 16384 candidates, and every
    128-mention block exactly 4096 candidates.  No padding, NT=32 tiles,
    MC=4 blocks, and each candidate tile maps to exactly one block.
  * A = mention_embs @ W1_top is computed on the HOST (tiny GEMM) and fed
    per-core in bf16; the device adds it into the pre-relu PSUM via one
    one-hot "expansion" matmul per (jc, tile) (E is host-built, packed
    into the same DMA slab as the candidate chunks).
  * Everything on the matmul path is bf16 (tolerance 2e-2; bf16 ~4e-3):
    halves HBM traffic and the startup DMA wait.
  * relu(psum + b1[j]) on ACT -> bf16 ht; W2 reduction on PE (6 matmuls
    of [128,1] weights); W2(jc) is issued after jc+1's matmuls so the PE
    never waits on ACT.
  * scores -> DRAM scratch with per-block 64-elem zero tails, so each
    output block's ragged->padded gather (indirect DMA windows, masked,
    + 0.5*faiss, both host-precomputed) depends only on its own chunk.
  * A short burst of warm-up matmuls runs during the initial DMA wait to
    lift the HAM clock gate (PE starts at 1.2 GHz otherwise).
"""

import sys

sys.path.insert(0, "/opt/trn_rl_repo")

from contextlib import ExitStack

import numpy as np
import ml_dtypes

import concourse.bass as bass
import concourse.tile as tile
from concourse import mybir
from concourse.tile_rust import add_dep_helper

F32 = mybir.dt.float32
BF16 = mybir.dt.bfloat16
I32 = mybir.dt.int32
AF = mybir.ActivationFunctionType
ALU = mybir.AluOpType
BF16NP = ml_dtypes.bfloat16

N_CORES = 8
H = 768
P = 128
KC = H // P            # 6 k-chunks per 768
JC = H // P            # 6 j-chunks
MAXK = 64
TT = 512               # candidate tile size
N_WARM = 64            # warm-up matmuls to lift the HAM clock gate and keep
                       # the PE busy until the first candidate slab lands


class SplitDrainTileContext(tile.TileContext):
    """The tail drain would carry one sync wait per logical proc; walrus caps
    sync waits per instruction. Absorb the global clock one proc at a time
    through SP NOPs (<=1 wait each), then emit the drain with a zero clock."""

    def _drain_and_barrier(self, tick_clock, wait_clock):
        from concourse.vector_clock import ScopedClock, VectorClock

        vals = list(tick_clock.global_clock)
        nprocs = len(vals)
        for q in range(nprocs):
            if not vals[q]:
                continue
            partial = [vals[p] if p == q else 0 for p in range(nprocs)]
            nop = self.nc.sync.nop()
            wait_clock.add_sem_waits(
                nop.ins, ScopedClock({None: VectorClock(partial)})
            )
        drain_inst = self.nc.sync.drain()
        wait_clock.add_sem_waits(
            drain_inst.ins, ScopedClock({None: VectorClock([0] * nprocs)})
        )
        self.nc.all_engine_barrier()
        popped = self.nc._tile_sem_poison_stack.pop()
        assert popped is self._sem_poison
        self.nc.clear_and_free_semaphores(list(self.sems.allocated().values()))
        self.nc.all_engine_barrier()


def split_waits(nc, cap=1):
    """This walrus build allows only ONE sync wait per instruction (two for
    some structs, but one is universally safe).  Move extra waits onto
    freshly inserted same-engine NOPs placed right before the instruction —
    the engine stalls at the NOP instead, semantics unchanged."""
    for fn in nc.m.functions:
        for bb in fn.blocks:
            new = []
            for inst in bb.instructions:
                si = inst.sync_info
                waits = list(si.on_wait) if si and si.on_wait else []
                if len(waits) > cap:
                    keep = waits[-cap:]
                    for k, wt in enumerate(waits[:-cap]):
                        nop = mybir.InstNoOp(
                            name=f"{inst.name}-wsp{k}",
                            engine=inst.engine,
                            ins=[], outs=[],
                            sync_info=mybir.SyncInfo(on_wait=[wt], on_update=[]),
                        )
                        nc.register_instruction(nop)
                        new.append(nop)
                    inst.sync_info = mybir.SyncInfo(
                        on_wait=keep, on_update=list(si.on_update or [])
                    )
                new.append(inst)
            bb.instructions = new


def build_program(NT, MC, windows, gdep, aligned, T_pad):
    """One SPMD Bass program shared by all cores.

    windows[i]: sorted local m-chunk ids present in candidate tile i on ANY
    core (union).  gdep[mc]: scores-chunk index that must land before output
    block mc can be gathered.  aligned=True means chunk==block with per-block
    zero tails in the scratch.
    """
    assert len(windows) == NT
    assert len(gdep) == MC
    CHT = NT // MC if aligned else 8
    n_chunks = -(-NT // CHT)
    CH = CHT * TT
    stride = CH + 64 if aligned else CH
    slen = (n_chunks - 1) * stride + CH + 64 if aligned else T_pad + MAXK
    slab_cols = [(KC + len(windows[i])) * TT for i in range(NT)]
    slab_base = np.concatenate([[0], np.cumsum(slab_cols)]).astype(int)

    nc = bass.Bass()

    candE = nc.dram_tensor("candE", [P, int(slab_base[-1])], BF16,
                           kind="ExternalInput")
    w1b = nc.dram_tensor("w1b", [P, KC * H], BF16, kind="ExternalInput")
    a_t = nc.dram_tensor("a", [P, MC * H], BF16, kind="ExternalInput")
    w2 = nc.dram_tensor("w2", [P, JC], BF16, kind="ExternalInput")
    b1 = nc.dram_tensor("b1", [P, JC], F32, kind="ExternalInput")
    b2 = nc.dram_tensor("b2", [1, 1], F32, kind="ExternalInput")
    warm = nc.dram_tensor("warm", [1, P], BF16, kind="ExternalInput")
    offs = nc.dram_tensor("offs", [P, MC], I32, kind="ExternalInput")
    maskf = nc.dram_tensor("maskf", [P, MC * MAXK], F32, kind="ExternalInput")
    fh = nc.dram_tensor("fh", [P, MC * MAXK], F32, kind="ExternalInput")

    out = nc.dram_tensor("out", [MC * P, MAXK], F32, kind="ExternalOutput")
    sc_dram = nc.dram_tensor("sc_scratch", [slen, 1], F32, kind="Internal")
    sc_flat = sc_dram[:].rearrange("t a -> (a t)")[None, :]

    with ExitStack() as ctx:
        tc = ctx.enter_context(SplitDrainTileContext(nc))
        cst = ctx.enter_context(tc.tile_pool(name="cst", bufs=1))
        candp = ctx.enter_context(tc.tile_pool(name="candp", bufs=4))
        htp = ctx.enter_context(tc.tile_pool(name="htp", bufs=8))
        gp = ctx.enter_context(tc.tile_pool(name="gp", bufs=2))
        scp = ctx.enter_context(tc.tile_pool(name="scp", bufs=2))
        hps = ctx.enter_context(tc.tile_pool(name="hps", bufs=6, space="PSUM"))
        sps = ctx.enter_context(tc.tile_pool(name="sps", bufs=1, space="PSUM"))
        wps_pool = ctx.enter_context(
            tc.tile_pool(name="wps", bufs=1, space="PSUM"))

        # ---- constants; critical startup DMAs (warm, w1b, slab0 below, a)
        # go in priority order on the sync queue, the small/late-needed ones
        # on the gpsimd queue in parallel ----
        warm_sb = cst.tile([1, P], BF16)
        nc.sync.dma_start(warm_sb[:], warm[:])
        w1b_sb = cst.tile([P, KC * H], BF16)
        nc.sync.dma_start(w1b_sb[:], w1b[:])
        a_sb = cst.tile([P, MC * H], BF16)
        nc.scalar.dma_start(a_sb[:], a_t[:])
        w2_sb = cst.tile([P, JC], BF16)
        nc.gpsimd.dma_start(w2_sb[:], w2[:])
        b1_sb = cst.tile([P, JC], F32)
        nc.gpsimd.dma_start(b1_sb[:], b1[:])
        b2_sb = cst.tile([1, 1], F32)
        nc.gpsimd.dma_start(b2_sb[:], b2[:])
        offs_sb = cst.tile([P, MC], I32)
        nc.gpsimd.dma_start(offs_sb[:], offs[:])
        maskf_sb = cst.tile([P, MC * MAXK], F32)
        nc.gpsimd.dma_start(maskf_sb[:], maskf[:])
        fh_sb = cst.tile([P, MC * MAXK], F32)
        nc.gpsimd.dma_start(fh_sb[:], fh[:])

        # zero tails of the scores scratch (gather windows overread 64)
        z_t = cst.tile([1, MAXK], F32)
        nc.vector.memset(z_t[:], 0.0)
        zero_dmas = []
        if aligned:
            for c in range(n_chunks):
                zero_dmas.append(nc.gpsimd.dma_start(
                    sc_flat[0:1, c * stride + CH:c * stride + CH + MAXK],
                    z_t[0:1, :]))
        else:
            zero_dmas.append(nc.gpsimd.dma_start(
                sc_flat[0:1, T_pad:T_pad + MAXK], z_t[0:1, :]))

        # ---- warm-up spinner: lift the HAM clock gate during DMA wait ----
        wps = wps_pool.tile([P, P], F32, tag="warmps")
        for k in range(N_WARM):
            nc.tensor.matmul(
                wps[:], lhsT=warm_sb[:], rhs=warm_sb[:],
                start=(k == 0), stop=(k == N_WARM - 1),
            )

        # ---- output stage: gather + mask + faiss for block mc ----
        def emit_out_chunk(mc, dep_dmas):
            g_t = gp.tile([P, MAXK], F32, tag="gath")
            gth = nc.gpsimd.indirect_dma_start(
                out=g_t[:], out_offset=None,
                in_=sc_dram[:],
                in_offset=bass.IndirectOffsetOnAxis(
                    ap=offs_sb[:, mc:mc + 1], axis=0),
            )
            for d in dep_dmas:
                add_dep_helper(gth.ins, d.ins, reason="gather needs scores")
            gm_t = gp.tile([P, MAXK], F32, tag="gm")
            nc.vector.tensor_tensor(
                gm_t[:], g_t[:], maskf_sb[:, mc * MAXK:(mc + 1) * MAXK],
                ALU.mult)
            o_t = gp.tile([P, MAXK], F32, tag="osb")
            nc.vector.tensor_tensor(
                o_t[:], gm_t[:], fh_sb[:, mc * MAXK:(mc + 1) * MAXK], ALU.add)
            nc.sync.dma_start(out[mc * P:(mc + 1) * P, :], o_t[:])

        # ---- main loop ----
        # Tile i's six W2 matmuls are BUNDLED and issued after tile i+1's
        # first group: by then all six relus are long done, so the bundle
        # carries a single (satisfied) ACT wait instead of six — a matmul
        # with an attached sync wait costs ~+100ns on the PE.  The bundle's
        # wait also subsumes all later hps/sps slot waits of the tile.
        sc_tiles = {}                 # chunk idx -> sc tile
        pend_w2 = None                # (s_ps, ht_tiles) of previous tile
        pend_sc = None                # (s_ps, tile idx) awaiting score copy

        def flush_w2():
            nonlocal pend_w2
            if pend_w2 is not None:
                s_ps, hts = pend_w2
                for jc in range(JC):
                    nc.tensor.matmul(
                        s_ps[0:1, :], lhsT=w2_sb[:, jc:jc + 1],
                        rhs=hts[jc][:],
                        start=(jc == 0), stop=(jc == JC - 1),
                    )
                pend_w2 = None

        def flush_sc():
            nonlocal pend_sc
            if pend_sc is not None:
                s_ps, i = pend_sc
                ci = i // CHT
                sc_t = sc_tiles[ci]
                nc.scalar.activation(
                    sc_t[0:1, (i % CHT) * TT:(i % CHT) * TT + TT],
                    s_ps[0:1, :], AF.Identity, bias=b2_sb[0:1, 0:1],
                )
                pend_sc = None
                if i % CHT == CHT - 1 or i == NT - 1:
                    c0 = ci * stride
                    cn = min(CH, T_pad - ci * CH)
                    d = nc.sync.dma_start(
                        sc_flat[0:1, c0:c0 + cn], sc_t[0:1, 0:cn]
                    )
                    del sc_tiles[ci]
                    for mc in range(MC):
                        if gdep[mc] == ci:
                            emit_out_chunk(mc, [d] + zero_dmas)

        for i in range(NT):
            if i % CHT == 0:
                sc_tiles[i // CHT] = scp.tile(
                    [1, CH], F32, tag="scchunk", name=f"sc_chunk{i // CHT}")
            cand_t = candp.tile([P, slab_cols[i]], BF16, tag="cand")
            nc.sync.dma_start(
                cand_t[:], candE[0:P, int(slab_base[i]):int(slab_base[i + 1])]
            )
            hts = []
            for jc in range(JC):
                ps = hps.tile([P, TT], F32, tag="hpsum")
                for kc in range(KC):
                    nc.tensor.matmul(
                        ps[:],
                        lhsT=w1b_sb[:, kc * H + jc * P:kc * H + (jc + 1) * P],
                        rhs=cand_t[:, kc * TT:(kc + 1) * TT],
                        start=(kc == 0), stop=False,
                    )
                nw = len(windows[i])
                for wi, w in enumerate(windows[i]):
                    nc.tensor.matmul(
                        ps[:],
                        lhsT=a_sb[:, w * H + jc * P:w * H + (jc + 1) * P],
                        rhs=cand_t[:, (KC + wi) * TT:(KC + wi + 1) * TT],
                        start=False, stop=(wi == nw - 1),
                    )
                if jc == 0:
                    # previous tile's W2 bundle + score copy, then its sps
                    # slot is free for this tile
                    flush_w2()
                    flush_sc()
                    s_ps = sps.tile([1, TT], F32, tag="spsum")
                ht_t = htp.tile([P, TT], BF16, tag="ht")
                nc.scalar.activation(
                    ht_t[:], ps[:], AF.Relu, bias=b1_sb[:, jc:jc + 1]
                )
                hts.append(ht_t)
            pend_w2 = (s_ps, hts)
            pend_sc = (s_ps, i)

        flush_w2()
        flush_sc()

    split_waits(nc)
    return nc


def _pair_mentions(lengths):
    """Match mentions into pairs with length sum exactly 64 (the generator
    pairs 32+d with 32-d).  Returns [n_pairs, 2] global ids or None."""
    n = len(lengths)
    if n % 2:
        return None
    order = np.argsort(lengths, kind="stable")
    lo, hi = 0, n - 1
    pairs = []
    while lo < hi:
        a, b = order[lo], order[hi]
        if lengths[a] + lengths[b] != 64:
            return None
        pairs.append((a, b))
        lo += 1
        hi -= 1
    return np.asarray(pairs, dtype=np.int64)


def prepare(inputs):
    """Shard + lay out the full inputs; returns (build params, in_maps, meta)."""
    mention_embs = np.asarray(inputs["mention_embs"], dtype=np.float32)
    candidate_embs = np.asarray(inputs["candidate_embs"], dtype=np.float32)
    W1 = np.asarray(inputs["W1"], dtype=np.float32)
    b1 = np.asarray(inputs["b1"], dtype=np.float32)
    W2 = np.asarray(inputs["W2"], dtype=np.float32)
    b2 = np.asarray(inputs["b2"], dtype=np.float32)
    faiss_prior = np.asarray(inputs["faiss_prior"], dtype=np.float32)
    mention_idx = np.asarray(inputs["mention_idx"], dtype=np.int64)
    col_idx = np.asarray(inputs["col_idx"], dtype=np.int64)

    N = mention_embs.shape[0]
    T = mention_idx.shape[0]
    assert np.all(np.diff(mention_idx) >= 0), "mention_idx must be sorted"
    lengths = np.bincount(mention_idx, minlength=N)
    offsets = np.concatenate([[0], np.cumsum(lengths)[:-1]])
    assert np.array_equal(col_idx, np.arange(T) - np.repeat(offsets, lengths))

    pairs = _pair_mentions(lengths) if (N % (2 * N_CORES) == 0) else None
    if pairs is not None:
        # perfect split: 256 pairs -> 512 mentions / 16384 cands per core,
        # every 128 mentions (64 pairs) = exactly 4096 candidates
        ppc = pairs.shape[0] // N_CORES
        perms = [pairs[c * ppc:(c + 1) * ppc].reshape(-1)
                 for c in range(N_CORES)]
        aligned = True
    else:
        # fallback: greedy balance by candidate count
        order = np.argsort(-lengths, kind="stable")
        loads = np.zeros(N_CORES, dtype=np.int64)
        buckets = [[] for _ in range(N_CORES)]
        for m in order:
            c = int(np.argmin(loads))
            buckets[c].append(m)
            loads[c] += lengths[m]
        perms = [np.asarray(sorted(b), dtype=np.int64) for b in buckets]
        aligned = False

    T_cs = [int(lengths[p].sum()) for p in perms]
    M_cs = [len(p) for p in perms]
    T_pad = -(-max(T_cs) // TT) * TT
    M_pad = -(-max(M_cs) // P) * P
    NT, MC = T_pad // TT, M_pad // P
    CHT = NT // MC if aligned else 8
    n_chunks = -(-NT // CHT)
    CH = CHT * TT
    stride = CH + 64 if aligned else CH

    # host-side mention-part GEMM (tiny): A = mention_embs @ W1_top
    A = mention_embs @ W1[:H]

    windows = [set() for _ in range(NT)]
    core_data = []
    for c in range(N_CORES):
        perm = perms[c]
        T_c, M_c = T_cs[c], M_cs[c]
        lens_c = lengths[perm]
        offs_c = np.concatenate([[0], np.cumsum(lens_c)[:-1]])
        ml = np.full(T_pad, -1, dtype=np.int64)
        ml[:T_c] = np.repeat(np.arange(M_c), lens_c)
        for i in range(NT):
            seg = ml[i * TT:(i + 1) * TT]
            seg = seg[seg >= 0]
            if seg.size:
                for w in np.unique(seg // P):
                    windows[i].add(int(w))
        core_data.append((perm, T_c, M_c, lens_c, offs_c, ml))
    windows = [sorted(w) if w else [0] for w in windows]

    if aligned:
        gdep = list(range(MC))
        assert windows == [[i // CHT] for i in range(NT)]
    else:
        gdep = [0] * MC
        for c in range(N_CORES):
            perm, T_c, M_c, lens_c, offs_c, ml = core_data[c]
            for mc in range(MC):
                rows = offs_c[mc * P:(mc + 1) * P]
                if rows.size == 0:
                    continue
                end = min(int(rows.max()) + MAXK, T_pad)
                k = min((end - 1) // CH, n_chunks - 1)
                gdep[mc] = max(gdep[mc], k)

    slab_cols = [(KC + len(windows[i])) * TT for i in range(NT)]
    slab_base = np.concatenate([[0], np.cumsum(slab_cols)]).astype(int)

    # shared (replicated) tensors
    w1b_l = np.ascontiguousarray(
        W1[H:].reshape(KC, P, H).transpose(1, 0, 2).reshape(P, KC * H)
    ).astype(BF16NP)
    w2_l = np.ascontiguousarray(W2[:, 0].reshape(JC, P).T).astype(BF16NP)
    b1_l = np.ascontiguousarray(b1.reshape(JC, P).T)
    b2_l = b2.reshape(1, 1)
    warm_l = np.ones((1, P), dtype=BF16NP)
    iota64 = np.arange(MAXK, dtype=np.float32)[None, :]

    in_maps = []
    for c in range(N_CORES):
        perm, T_c, M_c, lens_c, offs_c, ml = core_data[c]
        # gather this core's candidate rows in core-local order
        sel = (np.repeat(offsets[perm] - offs_c, lens_c)
               + np.arange(T_c)) if M_c else np.zeros(0, dtype=np.int64)
        cand_core = candidate_embs[sel]                      # [T_c, H] f32
        candT = np.zeros((P, KC, T_pad), dtype=BF16NP)
        candT[:, :, :T_c] = cand_core.astype(BF16NP).T.reshape(
            KC, P, T_c).transpose(1, 0, 2)

        candE_l = np.zeros((P, int(slab_base[-1])), dtype=BF16NP)
        for i in range(NT):
            b0 = int(slab_base[i])
            candE_l[:, b0:b0 + KC * TT] = candT[
                :, :, i * TT:(i + 1) * TT].reshape(P, KC * TT)
            seg = ml[i * TT:(i + 1) * TT]
            for wi, w in enumerate(windows[i]):
                e = (seg[None, :] ==
                     (w * P + np.arange(P))[:, None]).astype(BF16NP)
                candE_l[:, b0 + (KC + wi) * TT:b0 + (KC + wi + 1) * TT] = e

        A_core = np.zeros((MC * P, H), dtype=np.float32)
        A_core[:M_c] = A[perm]
        a_l = np.ascontiguousarray(
            A_core.reshape(MC, P, H).transpose(1, 0, 2).reshape(P, MC * H)
        ).astype(BF16NP)

        offs_l = np.zeros(MC * P, dtype=np.int64)
        offs_l[:M_c] = offs_c
        if aligned:
            # per-block scratch regions are (CH + 64) apart
            offs_l[:M_c] = offs_c + 64 * (np.arange(M_c) // P)
        lens_l = np.zeros(MC * P, dtype=np.int64)
        lens_l[:M_c] = lens_c
        maskf_l = (iota64 < lens_l[:, None]).astype(np.float32)
        fh_l = np.zeros((MC * P, MAXK), dtype=np.float32)
        fh_l[:M_c] = 0.5 * faiss_prior[perm]

        in_maps.append({
            "candE": candE_l,
            "w1b": w1b_l, "a": a_l, "w2": w2_l,
            "b1": b1_l, "b2": b2_l, "warm": warm_l,
            "offs": np.ascontiguousarray(
                offs_l.reshape(MC, P).T).astype(np.int32),
            "maskf": np.ascontiguousarray(
                maskf_l.reshape(MC, P, MAXK).transpose(1, 0, 2)
                .reshape(P, MC * MAXK)),
            "fh": np.ascontiguousarray(
                fh_l.reshape(MC, P, MAXK).transpose(1, 0, 2)
                .reshape(P, MC * MAXK)),
        })
    return (NT, MC, windows, gdep, aligned, T_pad), in_maps, (perms, N)


def assemble(results, meta, nota_bias):
    perms, N = meta
    out = np.empty((N, MAXK + 1), dtype=np.float32)
    for c in range(N_CORES):
        out[perms[c], :MAXK] = results[c]["out"][:len(perms[c])]
    out[:, MAXK] = np.float32(nota_bias)
    return out


_CACHE = {}


def kernel(**inputs) -> np.ndarray:
    from concourse.bass_utils import run_bass_kernel_spmd

    key_params, in_maps, meta = prepare(inputs)
    NT, MC, windows, gdep, aligned, T_pad = key_params
    key = (NT, MC, tuple(tuple(w) for w in windows), tuple(gdep), aligned,
           T_pad)
    if key not in _CACHE:
        _CACHE[key] = build_program(NT, MC, windows, gdep, aligned, T_pad)
    nc = _CACHE[key]
    res = run_bass_kernel_spmd(nc, in_maps, list(range(N_CORES)))
    return assemble(res.results, meta, np.asarray(inputs["nota_bias"]))
